# revision 16
# baseline (speedup 1.0000x reference)
"""Multi-head attention with q/v LoRA on 8 trn2 NeuronCores (bf16).

Reference computation (B=2, N=2048, C=1024, H=16, HD=64, R=16):
    qkv = x @ w_qkv + b_qkv                -> split per-head q, k, v
    q  += ((q @ a_q) @ b_q) * 2.0          (per head; same for v)
    out = softmax(q k^T / 8) v             (full N x N scores)
    y   = out @ w_proj + b_proj

Sharding: tensor-parallel over heads -- each of the 8 cores owns 2 heads
(128 of the 3072 qkv columns) for both batches; the attention output is
resharded over tokens with a per-(batch,head-pair) AllToAll so each core
computes final proj rows for its 256 tokens per batch with the full
w_proj.

All matmul operands are bf16 (hardware runs fp32/f32r matmuls in 4-pass
fp32_mode=HIGH -- 4x slower); PSUM accumulation stays fp32.  Per core:
  1. DMA host-pretransposed bf16 x^T chunks straight to SBUF (no
     rounding pass), compute the qkv^T shard with weights stationary;
     bias-add epilogues on ACT (idle then) for batch 0, DVE for batch 1,
  2. LoRA via block-diagonal [128,32]/[32,128] bf16 matrices,
  3. per (batch, head, q-half): scores S^T = k^T' q^T -> exp on ACT
     (bf16 out) -> P @ [v | 1] accumulated in PSUM (ones column yields
     softmax sums); normalize with DVE reciprocal + PE ones-broadcast,
  4. AllToAll [8, 64, 256] bf16 per (batch, head-pair); receivers DMA
     the slots straight into the proj operand tile; proj accumulates
     w_proj chunks in PSUM, adds bias via a rank-1 matmul, and DMAs the
     result straight from PSUM to DRAM (no vector epilogue).
A dummy 32-byte AllToAll issued first absorbs the one-time collective
barrier under the qkv phase.  Batch 1's qkv/LoRA, the v transposes and
batch 0's proj are sliced into small "filler" closures emitted between
attention kt-steps so the PE queue never drains (the DVFS governor
drops the PE clock on idle); the last unit has no fillers so its
AllToAll fires as early as possible.
The host stitches the 8 token shards and transposes back to [B, N, C].
"""

import sys

sys.path.insert(0, "/opt/trn_rl_repo")
sys.path.insert(0, "/root/.axon_site")

import numpy as np
import ml_dtypes

import concourse.bass as bass
import concourse.mybir as mybir
import concourse.tile as tile
from concourse.bass_utils import run_bass_kernel_spmd

f32 = mybir.dt.float32
bf16 = mybir.dt.bfloat16
AF = mybir.ActivationFunctionType
NPBF16 = np.dtype(ml_dtypes.bfloat16)

B, N, C = 2, 2048, 1024
H, HD, R = 16, 64, 16
LORA_SCALE = 32.0 / R
ATTN_SCALE = HD ** -0.5
NCORES = 8
HPC = H // NCORES          # heads per core = 2
PC = HPC * HD              # partition columns per core = 128
ROWS = B * N               # 4096 tokens
RC = 512                   # row-chunk size for qkv production
NCH = N // RC              # 4 chunks per batch
TPC = N // NCORES          # tokens per core per batch = 256


def _legalize_waits(nc, max_waits=1):
    """This walrus build accepts at most one sync-wait per instruction;
    Tile attaches several.  Move surplus waits onto same-engine NoOps
    inserted immediately before the instruction (identical semantics)."""
    counter = 0
    for fn in nc.m.functions:
        for bb in fn.blocks:
            insts = bb.instructions
            out = []
            changed = False
            for inst in insts:
                si = inst.sync_info
                if si is not None and si.on_wait and len(si.on_wait) > max_waits:
                    waits = list(si.on_wait)
                    for w in waits[:-max_waits]:
                        counter += 1
                        nop = mybir.InstNoOp(
                            name=f"I-wfix-{counter}",
                            engine=inst.engine,
                            sync_info=mybir.SyncInfo(on_wait=[w], on_update=[]),
                        )
                        nc.register_instruction(nop)
                        out.append(nop)
                    si.on_wait.clear()
                    si.on_wait.extend(waits[-max_waits:])
                    changed = True
                out.append(inst)
            if changed:
                insts[:] = out


def build_nc():
    nc = bass.Bass(num_devices=NCORES)

    xt_d = nc.dram_tensor("xt", [C, ROWS], bf16, kind="ExternalInput")
    wq_d = nc.dram_tensor("wq", [128, 1024], bf16, kind="ExternalInput")
    wk_d = nc.dram_tensor("wk", [128, 1024], bf16, kind="ExternalInput")
    wv_d = nc.dram_tensor("wv", [128, 1024], bf16, kind="ExternalInput")
    bq_d = nc.dram_tensor("bq", [128, 1], f32, kind="ExternalInput")
    bk_d = nc.dram_tensor("bk", [128, 1], f32, kind="ExternalInput")
    bv_d = nc.dram_tensor("bv", [128, 1], f32, kind="ExternalInput")
    a2q_d = nc.dram_tensor("a2q", [128, 2 * R], bf16, kind="ExternalInput")
    b2q_d = nc.dram_tensor("b2q", [2 * R, 128], bf16, kind="ExternalInput")
    a2v_d = nc.dram_tensor("a2v", [128, 2 * R], bf16, kind="ExternalInput")
    b2v_d = nc.dram_tensor("b2v", [2 * R, 128], bf16, kind="ExternalInput")
    wp_d = nc.dram_tensor("wp", [128, 8 * 1024], bf16, kind="ExternalInput")
    bpb_d = nc.dram_tensor("bpb", [1, 1024], bf16, kind="ExternalInput")
    eye64x2_d = nc.dram_tensor("eye64x2", [128, 64], bf16, kind="ExternalInput")
    out_d = nc.dram_tensor("out", [B, C, TPC], f32, kind="ExternalOutput")

    with nc.allow_low_precision(
        reason="bf16 operands are intended; PSUM accumulation stays fp32"
    ), tile.TileContext(nc) as tc:
        with (
            tc.tile_pool(name="persist", bufs=1) as persist,
            tc.tile_pool(name="const", bufs=1) as const,
            tc.tile_pool(name="dram", bufs=1, space="DRAM") as dram,
            tc.tile_pool(name="xio", bufs=2) as xio_p,
            tc.tile_pool(name="work", bufs=2) as work_p,
            tc.tile_pool(name="ps", bufs=1, space="PSUM") as ps,
        ):
            qT = persist.tile([128, ROWS], bf16, tag="qT", name="qT")
            kT = persist.tile([128, ROWS], bf16, tag="kT", name="kT")
            vT = persist.tile([128, ROWS], bf16, tag="vT", name="vT")

            x_tiles = {}

            def issue_x(b, rci):
                t = xio_p.tile([128, 8 * RC], bf16, tag="xT", name=f"xT{b}{rci}")
                r0 = b * N + rci * RC
                nc.sync.dma_start(
                    out=t[:].rearrange("p (a r) -> p a r", a=8),
                    in_=xt_d[:, r0 : r0 + RC].rearrange("(a p) r -> p a r", p=128),
                )
                x_tiles[(b, rci)] = t
                return t

            # prefetch the first two x^T chunks' DMAs ahead of the weight
            # DMAs so qkv chunk 1 never waits on its data
            issue_x(0, 0)
            issue_x(0, 1)

            def loaded(name, dram_t, shape, dt=bf16):
                t = const.tile(list(shape), dt, tag=name, name=name)
                nc.sync.dma_start(out=t[:], in_=dram_t[:])
                return t

            w_t = [
                loaded("wq_t", wq_d, (128, 1024)),
                loaded("wk_t", wk_d, (128, 1024)),
                loaded("wv_t", wv_d, (128, 1024)),
            ]
            a2q_t = loaded("a2q_t", a2q_d, (128, 2 * R))
            b2q_t = loaded("b2q_t", b2q_d, (2 * R, 128))
            a2v_t = loaded("a2v_t", a2v_d, (128, 2 * R))
            b2v_t = loaded("b2v_t", b2v_d, (2 * R, 128))
            eye64x2 = loaded("eye64", eye64x2_d, (128, 64))
            bias_t = [
                loaded("bq", bq_d, (128, 1), f32),
                loaded("bk", bk_d, (128, 1), f32),
                loaded("bv", bv_d, (128, 1), f32),
            ]
            bpb_t = loaded("bpb", bpb_d, (1, 1024))

            ones_bf = const.tile([1, 256], bf16, tag="ones_bf", name="ones_bf")
            nc.gpsimd.memset(ones_bf[:], 1.0)
            ones_f = const.tile([1, 64], f32, tag="ones_f", name="ones_f")
            nc.gpsimd.memset(ones_f[:], 1.0)

            wp_t = const.tile([128, 8 * 1024], bf16, tag="wp_t", name="wp_t")

            # dummy 32B collective to absorb the one-time CC barrier
            dmy_i = dram.tile([8, 1, 2], bf16, tag="dmy_i", name="dmy_i")
            dmy_o = dram.tile([8, 1, 2], bf16, tag="dmy_o", name="dmy_o")
            nc.sync.dma_start(
                out=dmy_i[:].rearrange("a p e -> p (a e)"), in_=ones_bf[:, 0:16]
            )
            nc.gpsimd.collective_compute(
                "AllToAll",
                mybir.AluOpType.bypass,
                replica_groups=[list(range(NCORES))],
                ins=[dmy_i[:].opt()],
                outs=[dmy_o[:].opt()],
            )

            qkvT = (qT, kT, vT)

            def emit_qkv_m(b, rci, m, act_store):
                r0 = b * N + rci * RC
                xT_t = x_tiles[(b, rci)]
                acc = ps.tile([128, RC], f32, tag="acc", bufs=2, name=f"ac{b}{rci}{m}")
                for ci in range(8):
                    nc.tensor.matmul(
                        acc[:],
                        w_t[m][:, ci * 128 : (ci + 1) * 128],
                        xT_t[:, ci * RC : (ci + 1) * RC],
                        start=(ci == 0),
                        stop=(ci == 7),
                    )
                dst = qkvT[m][:, r0 : r0 + RC]
                if act_store:
                    nc.scalar.activation(dst, acc[:], AF.Identity, bias=bias_t[m][:])
                else:
                    nc.vector.tensor_scalar_add(dst, acc[:], bias_t[m][:])

            def qkv_fillers(b, rci):
                """Three per-matrix closures for one 512-token chunk; the
                last one prefetches the x chunk two steps ahead."""
                def mk(m):
                    def go():
                        emit_qkv_m(b, rci, m, act_store=False)
                        if m == 2 and (b, rci + 2) not in x_tiles and rci + 2 < NCH:
                            issue_x(b, rci + 2)
                    return go
                return [mk(m) for m in range(3)]

            def lora_fillers(b, ch):
                boff = b * N
                fs = []
                for ti, (dstT, a2, b2) in enumerate(
                    ((qT, a2q_t, b2q_t), (vT, a2v_t, b2v_t))
                ):
                    sl = slice(boff + ch * 512, boff + (ch + 1) * 512)
                    state = {}

                    def s1(dstT=dstT, a2=a2, sl=sl, state=state, nm=f"l{b}{ch}{ti}"):
                        t_ps = ps.tile([2 * R, 512], f32, tag="acc", bufs=2, name=f"t{nm}")
                        nc.tensor.matmul(t_ps[:], a2[:], dstT[:, sl], start=True, stop=True)
                        t_sb = work_p.tile([2 * R, 512], bf16, tag="lt", name=f"s{nm}")
                        nc.vector.tensor_copy(t_sb[:], t_ps[:])
                        state["t"] = t_sb

                    def s2(dstT=dstT, b2=b2, sl=sl, state=state, nm=f"l{b}{ch}{ti}"):
                        d_ps = ps.tile([128, 512], f32, tag="acc", bufs=2, name=f"d{nm}")
                        nc.tensor.matmul(d_ps[:], b2[:], state["t"][:], start=True, stop=True)
                        nc.vector.tensor_add(dstT[:, sl], dstT[:, sl], d_ps[:])

                    fs += [s1, s2]
                return fs

            def vaug_fillers(b, hl):
                """Quartered v^T->v transpose; quarter 0 allocates the tile
                and memsets the ones columns.  Returns (handle, fillers)."""
                boff = b * N
                hs = slice(hl * HD, (hl + 1) * HD)
                handle = {}

                def mk(q):
                    def go():
                        if q == 0:
                            va = work_p.tile(
                                [128, 16 * 65], bf16, tag="vaug", name=f"va{b}{hl}"
                            )
                            nc.gpsimd.memset(va[:], 1.0)  # ones survive at 64::65
                            handle["t"] = va
                        va = handle["t"]
                        for kt in range(4 * q, 4 * q + 4):
                            ko = boff + kt * 128
                            vtr = ps.tile(
                                [128, 64], bf16, tag="acc", bufs=2, name=f"vt{b}{hl}{kt}"
                            )
                            nc.tensor.transpose(
                                vtr[:], vT[hs, ko : ko + 128], eye64x2[hs, :]
                            )
                            nc.vector.tensor_copy(va[:, kt * 65 : kt * 65 + 64], vtr[:])
                    return go

                return handle, [mk(q) for q in range(4)]

            def proj_filler(b, recv_r, mt):
                def go():
                    y_ps = ps.tile([128, TPC], f32, tag="acc", bufs=2, name=f"y{b}{mt}")
                    for kc in range(8):
                        nc.tensor.matmul(
                            y_ps[:],
                            wp_t[:, kc * 1024 + mt * 128 : kc * 1024 + (mt + 1) * 128],
                            recv_r[:, kc * TPC : (kc + 1) * TPC],
                            start=(kc == 0),
                            stop=False,
                        )
                    # bias as a rank-1 outer product: bpb^T @ ones
                    nc.tensor.matmul(
                        y_ps[:],
                        bpb_t[:, mt * 128 : (mt + 1) * 128],
                        ones_bf[:],
                        start=False,
                        stop=True,
                    )
                    yst = work_p.tile([128, TPC], f32, tag="yst", bufs=3, name=f"yc{b}{mt}")
                    nc.vector.tensor_copy(yst[:], y_ps[:])
                    nc.gpsimd.dma_start(
                        out=out_d[b, mt * 128 : (mt + 1) * 128, :], in_=yst[:]
                    )
                return go

            def emit_unit(b, hl, qh, va_handle, a2a_in, fillers=(), defer_norm=True):
                boff = b * N
                hs = slice(hl * HD, (hl + 1) * HD)
                qoff = boff + qh * 1024
                o_ps = ps.tile([65, 1024], f32, tag="o", bufs=1, name=f"o{b}{hl}{qh}")
                fillers = list(fillers)
                fi = 0

                def emit_pv(p_tile, kt):
                    v_aug = va_handle["t"]
                    for qc in range(2):
                        nc.tensor.matmul(
                            o_ps[:, qc * 512 : (qc + 1) * 512],
                            v_aug[:, kt * 65 : kt * 65 + 65],
                            p_tile[:, qc * 512 : (qc + 1) * 512],
                            start=(kt == 0),
                            stop=(kt == 15),
                        )

                pending = None
                for kt in range(16):
                    ko = boff + kt * 128
                    s_ps = ps.tile([128, 1024], f32, tag="s", bufs=2, name=f"s{b}{hl}{qh}{kt}")
                    for qc in range(2):
                        nc.tensor.matmul(
                            s_ps[:, qc * 512 : (qc + 1) * 512],
                            kT[hs, ko : ko + 128],
                            qT[hs, qoff + qc * 512 : qoff + (qc + 1) * 512],
                            start=True,
                            stop=True,
                        )
                    p_sb = work_p.tile([128, 1024], bf16, tag="p", bufs=3, name=f"p{qh}{kt}")
                    nc.scalar.activation(p_sb[:], s_ps[:], AF.Exp, scale=ATTN_SCALE)
                    if pending is not None:
                        emit_pv(*pending)
                    pending = (p_sb, kt)
                    if kt % 2 == 1 and fi < len(fillers):
                        if fillers[fi] is not None:
                            fillers[fi]()
                        fi += 1
                emit_pv(*pending)
                while fi < len(fillers):
                    if fillers[fi] is not None:
                        fillers[fi]()
                    fi += 1
                # normalize: r = 1/sums as exp(-ln(sums)) on ACT -- ln and
                # exp share one activation table, the DVE reciprocal is
                # 7.6ns/elem and stalls the in-order PE queue at the
                # ones-broadcast.  r lands in bf16 so the broadcast matmul
                # runs at bf16 rate; the mul reads it straight from PSUM.
                ln_s = work_p.tile([1, 1024], f32, tag="lns", bufs=2, name=f"ln{hl}{qh}")
                nc.scalar.activation(ln_s[:], o_ps[64:65, :], AF.Ln)
                r_sb = work_p.tile([1, 1024], bf16, tag="r", bufs=2, name=f"r{b}{hl}{qh}")
                nc.scalar.activation(r_sb[:], ln_s[:], AF.Exp, scale=-1.0)
                nst = work_p.tile([65, 1024], f32, tag="nst", bufs=2, name=f"n{hl}{qh}")
                nc.vector.tensor_copy(nst[:], o_ps[:])

                def finish():
                    # broadcast r and scale; deferred into the next unit's
                    # filler stream so the bc matmul never stalls the PE
                    # queue waiting for the ACT-computed reciprocal
                    ost = work_p.tile([64, 1024], bf16, tag="ost", bufs=2, name=f"os{hl}{qh}")
                    for qc in range(2):
                        bc_ps = ps.tile([64, 512], f32, tag="acc", bufs=2, name=f"bc{qc}")
                        nc.tensor.matmul(
                            bc_ps[:],
                            ones_bf[:, 0:64],
                            r_sb[:, qc * 512 : (qc + 1) * 512],
                            start=True,
                            stop=True,
                        )
                        nc.vector.tensor_mul(
                            ost[:, qc * 512 : (qc + 1) * 512],
                            nst[0:64, qc * 512 : (qc + 1) * 512],
                            bc_ps[:],
                        )
                    for tci in range(4):
                        nc.sync.dma_start(
                            out=a2a_in[qh * 4 + tci, :, :],
                            in_=ost[:, tci * TPC : (tci + 1) * TPC],
                        )

                if defer_norm:
                    return finish
                finish()
                return None

            def emit_a2a(b, hl, a2a_in):
                a2a_out = dram.tile(
                    [8, 64, TPC], bf16, tag=f"ao{b}{hl}", name=f"ao{b}{hl}"
                )
                nc.gpsimd.collective_compute(
                    "AllToAll",
                    mybir.AluOpType.bypass,
                    replica_groups=[list(range(NCORES))],
                    ins=[a2a_in[:].opt()],
                    outs=[a2a_out[:].opt()],
                )
                return a2a_out

            def new_a2a_in(b, hl):
                return dram.tile([8, 64, TPC], bf16, tag=f"ai{b}{hl}", name=f"ai{b}{hl}")

            recv_tiles = {}

            def get_recv(b):
                if b not in recv_tiles:
                    recv_tiles[b] = work_p.tile(
                        [128, 8 * TPC], bf16, tag=f"rcr{b}", bufs=1, name=f"rr{b}"
                    )
                return recv_tiles[b]

            def emit_recv_head(b, hl, a2a_out):
                recv_r = get_recv(b)
                for kc in range(8):
                    nc.gpsimd.dma_start(
                        out=recv_r[hl * 64 : (hl + 1) * 64, kc * TPC : (kc + 1) * TPC],
                        in_=a2a_out[kc],
                    )
                return recv_r

            # ---- emission schedule ---------------------------------------
            # batch-0 qkv + lora, stores on the otherwise-idle ACT engine
            for rci in range(NCH):
                if rci + 1 < NCH and (0, rci + 1) not in x_tiles:
                    issue_x(0, rci + 1)
                for m in range(3):
                    emit_qkv_m(0, rci, m, act_store=True)
                for f in lora_fillers(0, rci):
                    f()

            va00, va00_f = vaug_fillers(0, 0)
            for f in va00_f:
                f()
            issue_x(1, 0)
            issue_x(1, 1)

            ai = {(0, 0): new_a2a_in(0, 0), (0, 1): new_a2a_in(0, 1),
                  (1, 0): new_a2a_in(1, 0), (1, 1): new_a2a_in(1, 1)}
            ao = {}

            def fin_a2a(fin, b, hl):
                # a2a must be EMITTED after the deferred staging writes or
                # the collective races the slot data
                def go():
                    fin()
                    ao[(b, hl)] = emit_a2a(b, hl, ai[(b, hl)])
                return go

            fin = emit_unit(0, 0, 0, va00, ai[(0, 0)],
                            qkv_fillers(1, 0) + lora_fillers(1, 0))
            va01, va01_f = vaug_fillers(0, 1)
            fin = emit_unit(0, 0, 1, va00, ai[(0, 0)],
                            [fin] + qkv_fillers(1, 1) + lora_fillers(1, 1) + va01_f[:1])
            for f in va01_f[1:]:
                f()
            va10, va10_f = vaug_fillers(1, 0)
            fin = emit_unit(0, 1, 0, va01, ai[(0, 1)],
                            [fin_a2a(fin, 0, 0)] + qkv_fillers(1, 2) + lora_fillers(1, 2))
            fin = emit_unit(0, 1, 1, va01, ai[(0, 1)],
                            [fin] + qkv_fillers(1, 3) + lora_fillers(1, 3) + va10_f)
            # full bf16 w_proj (first needed by proj(0)) -- two spray DMAs
            nc.sync.dma_start(out=wp_t[:, 0:4096], in_=wp_d[:, 0:4096])
            nc.sync.dma_start(out=wp_t[:, 4096:8192], in_=wp_d[:, 4096:8192])

            va11, va11_f = vaug_fillers(1, 1)
            fin = emit_unit(1, 0, 0, va10, ai[(1, 0)],
                            [fin_a2a(fin, 0, 1)] + va11_f)
            # recv triggers (gpsimd) sit behind this unit's work so their
            # wait on the a2a-done semaphore can't stall the memsets the
            # next units need
            emit_recv_head(0, 0, ao[(0, 0)])
            recv0 = emit_recv_head(0, 1, ao[(0, 1)])
            fin = emit_unit(1, 0, 1, va10, ai[(1, 0)],
                            [fin, None, None]
                            + [proj_filler(0, recv0, mt) for mt in range(0, 4)])
            fin = emit_unit(1, 1, 0, va11, ai[(1, 1)],
                            [fin_a2a(fin, 1, 0)]
                            + [proj_filler(0, recv0, mt) for mt in range(4, 8)])
            emit_recv_head(1, 0, ao[(1, 0)])
            emit_unit(1, 1, 1, va11, ai[(1, 1)], [fin], defer_norm=False)
            ao[(1, 1)] = emit_a2a(1, 1, ai[(1, 1)])
            recv1 = emit_recv_head(1, 1, ao[(1, 1)])
            for mt in range(8):
                proj_filler(1, recv1, mt)()

    _legalize_waits(nc)
    # populate .instr bytes for extended-ISA instructions; raw
    # run_bass_kernel skips Bacc.compile() which normally runs this
    mybir.codegen_inst_isa_subclasses(nc)
    return nc


_NC_CACHE = None


def _get_nc():
    global _NC_CACHE
    if _NC_CACHE is None:
        _NC_CACHE = build_nc()
    return _NC_CACHE


def _make_in_maps(inputs):
    x = np.ascontiguousarray(np.asarray(inputs["x"], dtype=np.float32)).reshape(ROWS, C)
    xt = np.ascontiguousarray(x.T).astype(NPBF16)    # [C, ROWS]
    w_qkv = np.asarray(inputs["w_qkv"], dtype=np.float32)
    b_qkv = np.asarray(inputs["b_qkv"], dtype=np.float32)
    a_q = np.asarray(inputs["a_q"], dtype=np.float32)
    b_q = np.asarray(inputs["b_q"], dtype=np.float32)
    a_v = np.asarray(inputs["a_v"], dtype=np.float32)
    b_v = np.asarray(inputs["b_v"], dtype=np.float32)
    w_proj = np.asarray(inputs["w_proj"], dtype=np.float32)
    b_proj = np.asarray(inputs["b_proj"], dtype=np.float32)

    def blkdiag(m):
        z = np.zeros((2 * m.shape[0], 2 * m.shape[1]), dtype=np.float32)
        z[: m.shape[0], : m.shape[1]] = m
        z[m.shape[0] :, m.shape[1] :] = m
        return z

    a2q = blkdiag(a_q).astype(NPBF16)
    b2q = (blkdiag(b_q) * LORA_SCALE).astype(NPBF16)
    a2v = blkdiag(a_v).astype(NPBF16)
    b2v = (blkdiag(b_v) * LORA_SCALE).astype(NPBF16)
    eye64x2 = np.vstack([np.eye(64, dtype=np.float32)] * 2).astype(NPBF16)

    def warr(w):                              # [1024, n] -> [128, 8*n] chunk-major
        n = w.shape[1]
        return np.ascontiguousarray(
            w.reshape(8, 128, n).transpose(1, 0, 2).reshape(128, 8 * n)
        ).astype(NPBF16)

    wp_full = warr(w_proj)                    # [128, 8*1024]
    bpb = np.ascontiguousarray(b_proj.reshape(1, 1024)).astype(NPBF16)

    in_maps = []
    for c in range(NCORES):
        in_maps.append(
            {
                "xt": xt,
                "wq": warr(w_qkv[:, 0 * C + c * PC : 0 * C + (c + 1) * PC]),
                "wk": warr(w_qkv[:, 1 * C + c * PC : 1 * C + (c + 1) * PC]),
                "wv": warr(w_qkv[:, 2 * C + c * PC : 2 * C + (c + 1) * PC]),
                "bq": np.ascontiguousarray(b_qkv[0 * C + c * PC : 0 * C + (c + 1) * PC].reshape(128, 1)),
                "bk": np.ascontiguousarray(b_qkv[1 * C + c * PC : 1 * C + (c + 1) * PC].reshape(128, 1)),
                "bv": np.ascontiguousarray(b_qkv[2 * C + c * PC : 2 * C + (c + 1) * PC].reshape(128, 1)),
                "a2q": a2q,
                "b2q": b2q,
                "a2v": a2v,
                "b2v": b2v,
                "wp": wp_full,
                "bpb": bpb,
                "eye64x2": eye64x2,
            }
        )
    return in_maps


def run_sharded(inputs, trace=False, **kw):
    nc = _get_nc()
    in_maps = _make_in_maps(inputs)
    res = run_bass_kernel_spmd(nc, in_maps, list(range(NCORES)), trace=trace, **kw)
    # results[c]["out"]: [B, C, TPC] -- core c's token shard of final y^T
    yT = np.concatenate([res.results[c]["out"] for c in range(NCORES)], axis=2)
    out = np.ascontiguousarray(yT.transpose(0, 2, 1))  # [B, N, C]
    return out, res


def kernel(**inputs) -> np.ndarray:
    out, _ = run_sharded(inputs, trace=False)
    return out


# revision 17
# speedup vs baseline: 1.2322x; 1.2322x over previous
"""Multi-head attention with q/v LoRA on 8 trn2 NeuronCores (bf16).

Reference computation (B=2, N=2048, C=1024, H=16, HD=64, R=16):
    qkv = x @ w_qkv + b_qkv                -> split per-head q, k, v
    q  += ((q @ a_q) @ b_q) * 2.0          (per head; same for v)
    out = softmax(q k^T / 8) v             (full N x N scores)
    y   = out @ w_proj + b_proj

Sharding: tensor-parallel over heads -- each of the 8 cores owns 2 heads
(128 of the 3072 qkv columns) for both batches; the attention output is
resharded over tokens with a per-(batch,head-pair) AllToAll so each core
computes final proj rows for its 256 tokens per batch with the full
w_proj.

All matmul operands are bf16 (hardware runs fp32/f32r matmuls in 4-pass
fp32_mode=HIGH -- 4x slower); PSUM accumulation stays fp32.  Per core:
  1. DMA host-pretransposed bf16 x^T chunks straight to SBUF (no
     rounding pass), compute the qkv^T shard with weights stationary;
     bias-add epilogues on ACT (idle then) for batch 0, DVE for batch 1,
  2. LoRA via block-diagonal [128,32]/[32,128] bf16 matrices,
  3. per (batch, head, q-half): scores S^T = k^T' q^T -> exp on ACT
     (bf16 out) -> P @ [v | 1] accumulated in PSUM (ones column yields
     softmax sums); normalize with DVE reciprocal + PE ones-broadcast,
  4. AllToAll [8, 64, 256] bf16 per (batch, head-pair); receivers DMA
     the slots straight into the proj operand tile; proj accumulates
     w_proj chunks in PSUM, adds bias via a rank-1 matmul, and DMAs the
     result straight from PSUM to DRAM (no vector epilogue).
A dummy 32-byte AllToAll issued first absorbs the one-time collective
barrier under the qkv phase.  Batch 1's qkv/LoRA, the v transposes and
batch 0's proj are sliced into small "filler" closures emitted between
attention kt-steps so the PE queue never drains (the DVFS governor
drops the PE clock on idle); the last unit has no fillers so its
AllToAll fires as early as possible.
The host stitches the 8 token shards and transposes back to [B, N, C].
"""

import sys

sys.path.insert(0, "/opt/trn_rl_repo")
sys.path.insert(0, "/root/.axon_site")

import numpy as np
import ml_dtypes

import concourse.bass as bass
import concourse.mybir as mybir
import concourse.tile as tile
from concourse.bass_utils import run_bass_kernel_spmd

f32 = mybir.dt.float32
bf16 = mybir.dt.bfloat16
AF = mybir.ActivationFunctionType
NPBF16 = np.dtype(ml_dtypes.bfloat16)

B, N, C = 2, 2048, 1024
H, HD, R = 16, 64, 16
LORA_SCALE = 32.0 / R
ATTN_SCALE = HD ** -0.5
NCORES = 8
HPC = H // NCORES          # heads per core = 2
PC = HPC * HD              # partition columns per core = 128
ROWS = B * N               # 4096 tokens
RC = 512                   # row-chunk size for qkv production
NCH = N // RC              # 4 chunks per batch
TPC = N // NCORES          # tokens per core per batch = 256


def _legalize_waits(nc, max_waits=1):
    """This walrus build accepts at most one sync-wait per instruction;
    Tile attaches several.  Move surplus waits onto same-engine NoOps
    inserted immediately before the instruction (identical semantics)."""
    counter = 0
    for fn in nc.m.functions:
        for bb in fn.blocks:
            insts = bb.instructions
            out = []
            changed = False
            for inst in insts:
                si = inst.sync_info
                if si is not None and si.on_wait and len(si.on_wait) > max_waits:
                    waits = list(si.on_wait)
                    for w in waits[:-max_waits]:
                        counter += 1
                        nop = mybir.InstNoOp(
                            name=f"I-wfix-{counter}",
                            engine=inst.engine,
                            sync_info=mybir.SyncInfo(on_wait=[w], on_update=[]),
                        )
                        nc.register_instruction(nop)
                        out.append(nop)
                    si.on_wait.clear()
                    si.on_wait.extend(waits[-max_waits:])
                    changed = True
                out.append(inst)
            if changed:
                insts[:] = out


def build_nc():
    nc = bass.Bass(num_devices=NCORES)

    xt_d = nc.dram_tensor("xt", [C, ROWS], bf16, kind="ExternalInput")
    wq_d = nc.dram_tensor("wq", [128, 1024], bf16, kind="ExternalInput")
    wk_d = nc.dram_tensor("wk", [128, 1024], bf16, kind="ExternalInput")
    wv_d = nc.dram_tensor("wv", [128, 1024], bf16, kind="ExternalInput")
    bq_d = nc.dram_tensor("bq", [128, 1], f32, kind="ExternalInput")
    bk_d = nc.dram_tensor("bk", [128, 1], f32, kind="ExternalInput")
    bv_d = nc.dram_tensor("bv", [128, 1], f32, kind="ExternalInput")
    a2q_d = nc.dram_tensor("a2q", [128, 2 * R], bf16, kind="ExternalInput")
    b2q_d = nc.dram_tensor("b2q", [2 * R, 128], bf16, kind="ExternalInput")
    a2v_d = nc.dram_tensor("a2v", [128, 2 * R], bf16, kind="ExternalInput")
    b2v_d = nc.dram_tensor("b2v", [2 * R, 128], bf16, kind="ExternalInput")
    wp_d = nc.dram_tensor("wp", [128, 8 * 1024], bf16, kind="ExternalInput")
    bpb_d = nc.dram_tensor("bpb", [1, 1024], bf16, kind="ExternalInput")
    eye64x2_d = nc.dram_tensor("eye64x2", [128, 64], bf16, kind="ExternalInput")
    out_d = nc.dram_tensor("out", [B, C, TPC], f32, kind="ExternalOutput")

    with nc.allow_low_precision(
        reason="bf16 operands are intended; PSUM accumulation stays fp32"
    ), tile.TileContext(nc) as tc:
        with (
            tc.tile_pool(name="persist", bufs=1) as persist,
            tc.tile_pool(name="const", bufs=1) as const,
            tc.tile_pool(name="dram", bufs=1, space="DRAM") as dram,
            tc.tile_pool(name="xio", bufs=2) as xio_p,
            tc.tile_pool(name="work", bufs=2) as work_p,
            tc.tile_pool(name="ps", bufs=1, space="PSUM") as ps,
        ):
            qT = persist.tile([128, ROWS], bf16, tag="qT", name="qT")
            kT = persist.tile([128, ROWS], bf16, tag="kT", name="kT")
            vT = persist.tile([128, ROWS], bf16, tag="vT", name="vT")

            x_tiles = {}

            def issue_x(b, rci):
                t = xio_p.tile([128, 8 * RC], bf16, tag="xT", name=f"xT{b}{rci}")
                r0 = b * N + rci * RC
                nc.sync.dma_start(
                    out=t[:].rearrange("p (a r) -> p a r", a=8),
                    in_=xt_d[:, r0 : r0 + RC].rearrange("(a p) r -> p a r", p=128),
                )
                x_tiles[(b, rci)] = t
                return t

            # prefetch the first two x^T chunks' DMAs ahead of the weight
            # DMAs so qkv chunk 1 never waits on its data
            issue_x(0, 0)
            issue_x(0, 1)

            def loaded(name, dram_t, shape, dt=bf16):
                t = const.tile(list(shape), dt, tag=name, name=name)
                nc.sync.dma_start(out=t[:], in_=dram_t[:])
                return t

            w_t = [
                loaded("wq_t", wq_d, (128, 1024)),
                loaded("wk_t", wk_d, (128, 1024)),
                loaded("wv_t", wv_d, (128, 1024)),
            ]
            a2q_t = loaded("a2q_t", a2q_d, (128, 2 * R))
            b2q_t = loaded("b2q_t", b2q_d, (2 * R, 128))
            a2v_t = loaded("a2v_t", a2v_d, (128, 2 * R))
            b2v_t = loaded("b2v_t", b2v_d, (2 * R, 128))
            eye64x2 = loaded("eye64", eye64x2_d, (128, 64))
            bias_t = [
                loaded("bq", bq_d, (128, 1), f32),
                loaded("bk", bk_d, (128, 1), f32),
                loaded("bv", bv_d, (128, 1), f32),
            ]
            bpb_t = loaded("bpb", bpb_d, (1, 1024))

            ones_bf = const.tile([1, 256], bf16, tag="ones_bf", name="ones_bf")
            nc.gpsimd.memset(ones_bf[:], 1.0)
            ones_f = const.tile([1, 64], f32, tag="ones_f", name="ones_f")
            nc.gpsimd.memset(ones_f[:], 1.0)

            wp_t = const.tile([128, 8 * 1024], bf16, tag="wp_t", name="wp_t")

            # dummy 32B collective to absorb the one-time CC barrier
            dmy_i = dram.tile([8, 1, 2], bf16, tag="dmy_i", name="dmy_i")
            dmy_o = dram.tile([8, 1, 2], bf16, tag="dmy_o", name="dmy_o")
            nc.sync.dma_start(
                out=dmy_i[:].rearrange("a p e -> p (a e)"), in_=ones_bf[:, 0:16]
            )
            nc.gpsimd.collective_compute(
                "AllToAll",
                mybir.AluOpType.bypass,
                replica_groups=[list(range(NCORES))],
                ins=[dmy_i[:].opt()],
                outs=[dmy_o[:].opt()],
            )

            qkvT = (qT, kT, vT)

            def emit_qkv_m(b, rci, m, act_store):
                r0 = b * N + rci * RC
                xT_t = x_tiles[(b, rci)]
                acc = ps.tile([128, RC], f32, tag="acc", bufs=2, name=f"ac{b}{rci}{m}")
                for ci in range(8):
                    nc.tensor.matmul(
                        acc[:],
                        w_t[m][:, ci * 128 : (ci + 1) * 128],
                        xT_t[:, ci * RC : (ci + 1) * RC],
                        start=(ci == 0),
                        stop=(ci == 7),
                    )
                dst = qkvT[m][:, r0 : r0 + RC]
                if act_store:
                    nc.scalar.activation(dst, acc[:], AF.Identity, bias=bias_t[m][:])
                else:
                    nc.vector.tensor_scalar_add(dst, acc[:], bias_t[m][:])

            def qkv_fillers(b, rci):
                """Three per-matrix closures for one 512-token chunk; the
                last one prefetches the x chunk two steps ahead."""
                def mk(m):
                    def go():
                        emit_qkv_m(b, rci, m, act_store=False)
                        if m == 2 and (b, rci + 2) not in x_tiles and rci + 2 < NCH:
                            issue_x(b, rci + 2)
                    return go
                return [mk(m) for m in range(3)]

            def lora_fillers(b, ch):
                boff = b * N
                fs = []
                for ti, (dstT, a2, b2) in enumerate(
                    ((qT, a2q_t, b2q_t), (vT, a2v_t, b2v_t))
                ):
                    sl = slice(boff + ch * 512, boff + (ch + 1) * 512)
                    state = {}

                    def s1(dstT=dstT, a2=a2, sl=sl, state=state, nm=f"l{b}{ch}{ti}"):
                        t_ps = ps.tile([2 * R, 512], f32, tag="acc", bufs=2, name=f"t{nm}")
                        nc.tensor.matmul(t_ps[:], a2[:], dstT[:, sl], start=True, stop=True)
                        t_sb = work_p.tile([2 * R, 512], bf16, tag="lt", name=f"s{nm}")
                        nc.vector.tensor_copy(t_sb[:], t_ps[:])
                        state["t"] = t_sb

                    def s2(dstT=dstT, b2=b2, sl=sl, state=state, nm=f"l{b}{ch}{ti}"):
                        d_ps = ps.tile([128, 512], f32, tag="acc", bufs=2, name=f"d{nm}")
                        nc.tensor.matmul(d_ps[:], b2[:], state["t"][:], start=True, stop=True)
                        nc.vector.tensor_add(dstT[:, sl], dstT[:, sl], d_ps[:])

                    fs += [s1, s2]
                return fs

            def vaug_fillers(b, hl):
                """Quartered v^T->v transpose; quarter 0 allocates the tile
                and memsets the ones columns.  Returns (handle, fillers)."""
                boff = b * N
                hs = slice(hl * HD, (hl + 1) * HD)
                handle = {}

                def mk(q):
                    def go():
                        if q == 0:
                            va = work_p.tile(
                                [128, 16 * 65], bf16, tag="vaug", name=f"va{b}{hl}"
                            )
                            nc.gpsimd.memset(va[:], 1.0)  # ones survive at 64::65
                            handle["t"] = va
                        va = handle["t"]
                        for kt in range(4 * q, 4 * q + 4):
                            ko = boff + kt * 128
                            vtr = ps.tile(
                                [128, 64], bf16, tag="acc", bufs=2, name=f"vt{b}{hl}{kt}"
                            )
                            nc.tensor.transpose(
                                vtr[:], vT[hs, ko : ko + 128], eye64x2[hs, :]
                            )
                            nc.vector.tensor_copy(va[:, kt * 65 : kt * 65 + 64], vtr[:])
                    return go

                return handle, [mk(q) for q in range(4)]

            def proj_filler(b, recv_r, mt):
                def go():
                    y_ps = ps.tile([128, TPC], f32, tag="acc", bufs=2, name=f"y{b}{mt}")
                    for kc in range(8):
                        nc.tensor.matmul(
                            y_ps[:],
                            wp_t[:, kc * 1024 + mt * 128 : kc * 1024 + (mt + 1) * 128],
                            recv_r[:, kc * TPC : (kc + 1) * TPC],
                            start=(kc == 0),
                            stop=False,
                        )
                    # bias as a rank-1 outer product: bpb^T @ ones
                    nc.tensor.matmul(
                        y_ps[:],
                        bpb_t[:, mt * 128 : (mt + 1) * 128],
                        ones_bf[:],
                        start=False,
                        stop=True,
                    )
                    yst = work_p.tile([128, TPC], f32, tag="yst", bufs=3, name=f"yc{b}{mt}")
                    nc.vector.tensor_copy(yst[:], y_ps[:])
                    nc.gpsimd.dma_start(
                        out=out_d[b, mt * 128 : (mt + 1) * 128, :], in_=yst[:]
                    )
                return go

            def emit_unit(b, hl, qh, va_handle, a2a_in, fillers=(), defer_norm=True):
                boff = b * N
                hs = slice(hl * HD, (hl + 1) * HD)
                qoff = boff + qh * 1024
                o_ps = ps.tile([65, 1024], f32, tag="o", bufs=1, name=f"o{b}{hl}{qh}")
                fillers = list(fillers)
                fi = 0

                def emit_pv(p_tile, kt):
                    v_aug = va_handle["t"]
                    for qc in range(2):
                        nc.tensor.matmul(
                            o_ps[:, qc * 512 : (qc + 1) * 512],
                            v_aug[:, kt * 65 : kt * 65 + 65],
                            p_tile[:, qc * 512 : (qc + 1) * 512],
                            start=(kt == 0),
                            stop=(kt == 15),
                        )

                pending = None
                for kt in range(16):
                    ko = boff + kt * 128
                    s_ps = ps.tile([128, 1024], f32, tag="s", bufs=2, name=f"s{b}{hl}{qh}{kt}")
                    for qc in range(2):
                        nc.tensor.matmul(
                            s_ps[:, qc * 512 : (qc + 1) * 512],
                            kT[hs, ko : ko + 128],
                            qT[hs, qoff + qc * 512 : qoff + (qc + 1) * 512],
                            start=True,
                            stop=True,
                        )
                    p_sb = work_p.tile([128, 1024], bf16, tag="p", bufs=3, name=f"p{qh}{kt}")
                    nc.scalar.activation(p_sb[:], s_ps[:], AF.Exp, scale=ATTN_SCALE)
                    if pending is not None:
                        emit_pv(*pending)
                    pending = (p_sb, kt)
                    if kt % 2 == 1 and fi < len(fillers):
                        if fillers[fi] is not None:
                            fillers[fi]()
                        fi += 1
                emit_pv(*pending)
                while fi < len(fillers):
                    if fillers[fi] is not None:
                        fillers[fi]()
                    fi += 1
                # normalize: r = 1/sums as exp(-ln(sums)) on ACT -- ln and
                # exp share one activation table, the DVE reciprocal is
                # 7.6ns/elem and stalls the in-order PE queue at the
                # ones-broadcast.  r lands in bf16 so the broadcast matmul
                # runs at bf16 rate; the mul reads it straight from PSUM.
                ln_s = work_p.tile([1, 1024], f32, tag="lns", bufs=2, name=f"ln{hl}{qh}")
                nc.scalar.activation(ln_s[:], o_ps[64:65, :], AF.Ln)
                r_sb = work_p.tile([1, 1024], bf16, tag="r", bufs=2, name=f"r{b}{hl}{qh}")
                nc.scalar.activation(r_sb[:], ln_s[:], AF.Exp, scale=-1.0)
                nst = work_p.tile([65, 1024], f32, tag="nst", bufs=2, name=f"n{hl}{qh}")
                nc.vector.tensor_copy(nst[:], o_ps[:])

                def finish():
                    # broadcast r and scale; deferred into the next unit's
                    # filler stream so the bc matmul never stalls the PE
                    # queue waiting for the ACT-computed reciprocal
                    ost = work_p.tile([64, 1024], bf16, tag="ost", bufs=2, name=f"os{hl}{qh}")
                    for qc in range(2):
                        bc_ps = ps.tile([64, 512], f32, tag="acc", bufs=2, name=f"bc{qc}")
                        nc.tensor.matmul(
                            bc_ps[:],
                            ones_bf[:, 0:64],
                            r_sb[:, qc * 512 : (qc + 1) * 512],
                            start=True,
                            stop=True,
                        )
                        nc.vector.tensor_mul(
                            ost[:, qc * 512 : (qc + 1) * 512],
                            nst[0:64, qc * 512 : (qc + 1) * 512],
                            bc_ps[:],
                        )
                    for tci in range(4):
                        nc.sync.dma_start(
                            out=a2a_in[qh * 4 + tci, :, :],
                            in_=ost[:, tci * TPC : (tci + 1) * TPC],
                        )

                if defer_norm:
                    return finish
                finish()
                return None

            def emit_a2a(b, hl, a2a_in):
                a2a_out = dram.tile(
                    [8, 64, TPC], bf16, tag=f"ao{b}{hl}", name=f"ao{b}{hl}"
                )
                nc.gpsimd.collective_compute(
                    "AllToAll",
                    mybir.AluOpType.bypass,
                    replica_groups=[list(range(NCORES))],
                    ins=[a2a_in[:].opt()],
                    outs=[a2a_out[:].opt()],
                )
                return a2a_out

            def new_a2a_in(b, hl):
                return dram.tile([8, 64, TPC], bf16, tag=f"ai{b}{hl}", name=f"ai{b}{hl}")

            recv_tiles = {}

            def get_recv(b):
                if b not in recv_tiles:
                    recv_tiles[b] = work_p.tile(
                        [128, 8 * TPC], bf16, tag=f"rcr{b}", bufs=1, name=f"rr{b}"
                    )
                return recv_tiles[b]

            def emit_recv_head(b, hl, a2a_out):
                recv_r = get_recv(b)
                for kc in range(8):
                    nc.gpsimd.dma_start(
                        out=recv_r[hl * 64 : (hl + 1) * 64, kc * TPC : (kc + 1) * TPC],
                        in_=a2a_out[kc],
                    )
                return recv_r

            # ---- emission schedule ---------------------------------------
            # batch-0 qkv + lora, stores on the otherwise-idle ACT engine
            for rci in range(NCH):
                if rci + 1 < NCH and (0, rci + 1) not in x_tiles:
                    issue_x(0, rci + 1)
                for m in range(3):
                    emit_qkv_m(0, rci, m, act_store=True)
                for f in lora_fillers(0, rci):
                    f()

            va00, va00_f = vaug_fillers(0, 0)
            for f in va00_f:
                f()
            issue_x(1, 0)
            issue_x(1, 1)

            ai = {(0, 0): new_a2a_in(0, 0), (0, 1): new_a2a_in(0, 1),
                  (1, 0): new_a2a_in(1, 0), (1, 1): new_a2a_in(1, 1)}
            ao = {}

            # per head-pair: the qh=0 unit's broadcast/staging is deferred
            # into the qh=1 unit's filler slots (hides the ACT reciprocal
            # latency); the qh=1 unit finishes inline so the AllToAll can
            # be emitted -- and fire -- immediately after it
            fin = emit_unit(0, 0, 0, va00, ai[(0, 0)],
                            qkv_fillers(1, 0) + lora_fillers(1, 0))
            va01, va01_f = vaug_fillers(0, 1)
            emit_unit(0, 0, 1, va00, ai[(0, 0)],
                      [fin] + qkv_fillers(1, 1) + lora_fillers(1, 1) + va01_f[:1],
                      defer_norm=False)
            ao[(0, 0)] = emit_a2a(0, 0, ai[(0, 0)])
            for f in va01_f[1:]:
                f()
            va10, va10_f = vaug_fillers(1, 0)
            fin = emit_unit(0, 1, 0, va01, ai[(0, 1)],
                            qkv_fillers(1, 2) + lora_fillers(1, 2))
            emit_unit(0, 1, 1, va01, ai[(0, 1)],
                      [fin] + qkv_fillers(1, 3) + lora_fillers(1, 3) + va10_f,
                      defer_norm=False)
            ao[(0, 1)] = emit_a2a(0, 1, ai[(0, 1)])
            # full bf16 w_proj (first needed by proj(0)) -- two spray DMAs
            nc.sync.dma_start(out=wp_t[:, 0:4096], in_=wp_d[:, 0:4096])
            nc.sync.dma_start(out=wp_t[:, 4096:8192], in_=wp_d[:, 4096:8192])

            va11, va11_f = vaug_fillers(1, 1)
            fin = emit_unit(1, 0, 0, va10, ai[(1, 0)], va11_f)
            # recv triggers (gpsimd) sit behind this unit's work so their
            # wait on the a2a-done semaphore can't stall the memsets the
            # next units need
            emit_recv_head(0, 0, ao[(0, 0)])
            recv0 = emit_recv_head(0, 1, ao[(0, 1)])
            emit_unit(1, 0, 1, va10, ai[(1, 0)],
                      [fin, None, None, None]
                      + [proj_filler(0, recv0, mt) for mt in range(0, 4)],
                      defer_norm=False)
            ao[(1, 0)] = emit_a2a(1, 0, ai[(1, 0)])
            fin = emit_unit(1, 1, 0, va11, ai[(1, 1)],
                            [proj_filler(0, recv0, mt) for mt in range(4, 8)])
            emit_recv_head(1, 0, ao[(1, 0)])
            emit_unit(1, 1, 1, va11, ai[(1, 1)], [fin], defer_norm=False)
            ao[(1, 1)] = emit_a2a(1, 1, ai[(1, 1)])
            recv1 = emit_recv_head(1, 1, ao[(1, 1)])
            for mt in range(8):
                proj_filler(1, recv1, mt)()

    _legalize_waits(nc)
    # populate .instr bytes for extended-ISA instructions; raw
    # run_bass_kernel skips Bacc.compile() which normally runs this
    mybir.codegen_inst_isa_subclasses(nc)
    return nc


_NC_CACHE = None


def _get_nc():
    global _NC_CACHE
    if _NC_CACHE is None:
        _NC_CACHE = build_nc()
    return _NC_CACHE


def _make_in_maps(inputs):
    x = np.ascontiguousarray(np.asarray(inputs["x"], dtype=np.float32)).reshape(ROWS, C)
    xt = np.ascontiguousarray(x.T).astype(NPBF16)    # [C, ROWS]
    w_qkv = np.asarray(inputs["w_qkv"], dtype=np.float32)
    b_qkv = np.asarray(inputs["b_qkv"], dtype=np.float32)
    a_q = np.asarray(inputs["a_q"], dtype=np.float32)
    b_q = np.asarray(inputs["b_q"], dtype=np.float32)
    a_v = np.asarray(inputs["a_v"], dtype=np.float32)
    b_v = np.asarray(inputs["b_v"], dtype=np.float32)
    w_proj = np.asarray(inputs["w_proj"], dtype=np.float32)
    b_proj = np.asarray(inputs["b_proj"], dtype=np.float32)

    def blkdiag(m):
        z = np.zeros((2 * m.shape[0], 2 * m.shape[1]), dtype=np.float32)
        z[: m.shape[0], : m.shape[1]] = m
        z[m.shape[0] :, m.shape[1] :] = m
        return z

    a2q = blkdiag(a_q).astype(NPBF16)
    b2q = (blkdiag(b_q) * LORA_SCALE).astype(NPBF16)
    a2v = blkdiag(a_v).astype(NPBF16)
    b2v = (blkdiag(b_v) * LORA_SCALE).astype(NPBF16)
    eye64x2 = np.vstack([np.eye(64, dtype=np.float32)] * 2).astype(NPBF16)

    def warr(w):                              # [1024, n] -> [128, 8*n] chunk-major
        n = w.shape[1]
        return np.ascontiguousarray(
            w.reshape(8, 128, n).transpose(1, 0, 2).reshape(128, 8 * n)
        ).astype(NPBF16)

    wp_full = warr(w_proj)                    # [128, 8*1024]
    bpb = np.ascontiguousarray(b_proj.reshape(1, 1024)).astype(NPBF16)

    in_maps = []
    for c in range(NCORES):
        in_maps.append(
            {
                "xt": xt,
                "wq": warr(w_qkv[:, 0 * C + c * PC : 0 * C + (c + 1) * PC]),
                "wk": warr(w_qkv[:, 1 * C + c * PC : 1 * C + (c + 1) * PC]),
                "wv": warr(w_qkv[:, 2 * C + c * PC : 2 * C + (c + 1) * PC]),
                "bq": np.ascontiguousarray(b_qkv[0 * C + c * PC : 0 * C + (c + 1) * PC].reshape(128, 1)),
                "bk": np.ascontiguousarray(b_qkv[1 * C + c * PC : 1 * C + (c + 1) * PC].reshape(128, 1)),
                "bv": np.ascontiguousarray(b_qkv[2 * C + c * PC : 2 * C + (c + 1) * PC].reshape(128, 1)),
                "a2q": a2q,
                "b2q": b2q,
                "a2v": a2v,
                "b2v": b2v,
                "wp": wp_full,
                "bpb": bpb,
                "eye64x2": eye64x2,
            }
        )
    return in_maps


def run_sharded(inputs, trace=False, **kw):
    nc = _get_nc()
    in_maps = _make_in_maps(inputs)
    res = run_bass_kernel_spmd(nc, in_maps, list(range(NCORES)), trace=trace, **kw)
    # results[c]["out"]: [B, C, TPC] -- core c's token shard of final y^T
    yT = np.concatenate([res.results[c]["out"] for c in range(NCORES)], axis=2)
    out = np.ascontiguousarray(yT.transpose(0, 2, 1))  # [B, N, C]
    return out, res


def kernel(**inputs) -> np.ndarray:
    out, _ = run_sharded(inputs, trace=False)
    return out
